# revision 1
# baseline (speedup 1.0000x reference)
"""Equivariant rotation conv for Trainium2, 8-core batch-parallel.

Computes: rotate a (128*8, 128, 3, 3) filter bank by 8 data-dependent angles
(bilinear resampling), run a 3x3 same-padded conv of x (16,128,128,128) with
all 8*128 rotated filters, then max over the 8 rotations -> (16,128,128,128).

Sharding: data-parallel over batch, 2 images per core; the filter bank and
rotation coefficients are replicated.  On device, per core:
  - the 9x9 bilinear mixing matrix per rotation (a pure function of the 8
    rot_alpha scalars, computed on host and shipped alongside the weights)
    is applied to the filter bank with batched broadcast multiply-adds on
    DVE -> rotated bf16 lhsT tiles [Cin, 9 taps, O]; rotation 0 always has
    angle 0, so it is just a cast on the ACT engine,
  - the conv runs as 9 shifted PE matmuls in bf16 (K=Cin=128 partitions,
    N=512 spatial) accumulated in f32 PSUM, one PSUM bank per 4 output
    rows, 8 output-channel chunks = 8 rotations,
  - a running elementwise max over the rotation chunks on DVE, with the
    final max fused with the per-slice output DMA,
  - the first three row blocks share one rotation loop so the DVE rotation
    pipeline stays ahead of the PE; steady state runs the PE gap-free at
    ~218 ns per 512-column matmul (~98% busy, ~96% MFU).
"""

import numpy as np


def _install_axon_hooks_shim():
    """Provide antenv.axon_hooks (NTFF profile hook) when the image's antenv
    lacks it, so run_bass_kernel_spmd(trace=True) works instead of crashing
    on import.  The hook drives NRT profiling via ctypes into the axon PJRT
    plugin, mirroring the boot-side installer."""
    import contextlib
    import ctypes
    import os
    import sys
    import types

    try:
        import antenv.axon_hooks  # noqa: F401

        return
    except ImportError:
        pass

    state = {"hook": None, "resolved": False}

    def _make_hook():
        so_path = os.environ.get("AXON_PJRT_SO", "/opt/axon/libaxon_pjrt.so")
        if not os.path.exists(so_path):
            return None
        lib = ctypes.CDLL(so_path)
        if not hasattr(lib, "axon_start_nrt_profile"):
            return None
        lib.axon_start_nrt_profile.argtypes = [
            ctypes.POINTER(ctypes.c_int64),
            ctypes.c_size_t,
        ]
        lib.axon_start_nrt_profile.restype = ctypes.c_int64
        lib.axon_stop_nrt_profile.argtypes = [ctypes.c_char_p]
        lib.axon_stop_nrt_profile.restype = ctypes.c_int64

        @contextlib.contextmanager
        def _hook(output_dir, device_ids):
            import jax

            jax.devices()
            if device_ids:
                ids = (ctypes.c_int64 * len(device_ids))(*device_ids)
                rc = lib.axon_start_nrt_profile(ids, len(device_ids))
            else:
                rc = lib.axon_start_nrt_profile(None, 0)
            if rc != 0:
                raise RuntimeError(f"axon_start_nrt_profile rc={rc}")
            try:
                yield
            finally:
                n = lib.axon_stop_nrt_profile(str(output_dir).encode())
                if n < 0:
                    raise RuntimeError(f"axon_stop_nrt_profile rc={n}")
                print(f"profile: {n} file(s) written to {output_dir}")

        return _hook

    mod = types.ModuleType("antenv.axon_hooks")

    def set_axon_ntff_profile_hook(h):
        state["hook"] = h
        state["resolved"] = True

    def get_axon_ntff_profile_hook():
        if not state["resolved"]:
            state["hook"] = _make_hook()
            state["resolved"] = True
        return state["hook"]

    mod.set_axon_ntff_profile_hook = set_axon_ntff_profile_hook
    mod.get_axon_ntff_profile_hook = get_axon_ntff_profile_hook
    sys.modules["antenv.axon_hooks"] = mod


_install_axon_hooks_shim()

import concourse.bass as bass
import concourse.mybir as mybir
from concourse import bacc
from concourse.bass_utils import run_bass_kernel_spmd
from concourse.tile import TileContext
from concourse.tile_rust import add_dep_helper

F32 = mybir.dt.float32
F32R = mybir.dt.float32r
BF16 = mybir.dt.bfloat16

B, CIN, H, W = 16, 128, 128, 128
R, O, K = 8, 128, 3
NCORES = 8
BL = B // NCORES  # images per core
RB = 32           # output rows per block
NS = RB // 4      # psum subtiles (4 rows = 512 cols) per block
NBLK = H // RB

# "bf16": matmul operands bf16 (f32 accumulate).  "f32r": float32r operands.
MM_DTYPE = "bf16"

_TRACE = False
LAST_RESULTS = None
_NC_CACHE = {}


def _rot_mats(rot_alpha):
    """Per-rotation 9x9 bilinear resampling matrices, matching the reference
    F.grid_sample(align_corners=True, zeros) tap logic exactly.

    M[r, p, q]: coefficient of original tap q = (qy*3+qx) in rotated tap
    p = (py*3+px)."""
    M = np.zeros((R, 9, 9), np.float64)
    lin = np.linspace(-1.0, 1.0, K)
    for r in range(R):
        ang = float(rot_alpha[r]) * (np.pi / 4.0) * r
        c, s = np.cos(ang), np.sin(ang)
        for a in range(K):          # output row (gy = lin[a])
            for b in range(K):      # output col (gx = lin[b])
                gx, gy = lin[b], lin[a]
                xs = c * gx - s * gy
                ys = s * gx + c * gy
                ix = (xs + 1.0) * 0.5 * (K - 1)
                iy = (ys + 1.0) * 0.5 * (K - 1)
                x0 = int(np.floor(ix))
                y0 = int(np.floor(iy))
                wx, wy = ix - x0, iy - y0
                p = a * K + b
                for yi, xi, wt in (
                    (y0, x0, (1 - wy) * (1 - wx)),
                    (y0, x0 + 1, (1 - wy) * wx),
                    (y0 + 1, x0, wy * (1 - wx)),
                    (y0 + 1, x0 + 1, wy * wx),
                ):
                    if 0 <= yi < K and 0 <= xi < K:
                        M[r, p, yi * K + xi] += wt
    return M.astype(np.float32)


def _build(mm_dtype):
    use_bf16 = mm_dtype == "bf16"
    mm_dt = BF16 if use_bf16 else F32

    nc = bacc.Bacc(trn_type="TRN2")
    xs = nc.dram_tensor("xs", [BL, CIN, H, W], F32, kind="ExternalInput")
    # wl[r, i, :1152] = weights (q, o); wl[r, i, 1152:1233] = M[r] coefficients
    # (replicated across i) so each rotation needs exactly one input DMA.
    wl = nc.dram_tensor("wl", [R, CIN, 9 * O + 81], F32, kind="ExternalInput")
    y = nc.dram_tensor("y", [BL, O, H, W], F32, kind="ExternalOutput")

    with TileContext(nc) as tc:
        with (
            tc.tile_pool(name="wsrc", bufs=1) as wpool,
            tc.tile_pool(name="wrot", bufs=1) as rpool,
            tc.tile_pool(name="rtmp", bufs=1) as tpool,
            tc.tile_pool(name="xio", bufs=1) as xpool,
            tc.tile_pool(name="accp", bufs=3) as apool,
            tc.tile_pool(name="psum", bufs=1, space="PSUM") as ppool,
        ):
            worig = []
            rotw = []
            for r in range(R):
                wsr = wpool.tile([128, 9 * O + 81], F32, name=f"wsr{r}", tag=f"wsr{r}")
                worig.append(wsr)
                rw = rpool.tile([128, 9, O], mm_dt, name=f"rotw{r}", tag=f"rotw{r}")
                rotw.append(rw)

            # PE warm-up: ~125 dependency-free matmuls on an uninitialized
            # scratch tile keep the PE busy from ~0.5us until the first real
            # matmul (~15us), so the HAM clock gate reaches 8/8 before real
            # work and the first conv chunks run at 2.4 GHz instead of 1.2.
            # Results land in the ps0 bank slot and are overwritten by the
            # first real start=True accumulation group.
            dum_lhs = wpool.tile([128, 128], mm_dt, name="dum_lhs", tag="dum")
            nc.gpsimd.memset(dum_lhs[:, :], 0.0)
            dum_ps = ppool.tile([128, 128], F32, name="dum_ps", tag="ps0")
            for _ in range(125):
                nc.tensor.matmul(
                    dum_ps[:, :], dum_lhs[:, :], dum_lhs[:, :],
                    start=True, stop=True,
                )

            last_rot_op = [None]

            def emit_rotate(r):
                # rotw[r][i, p, o] = sum_q M[r,p,q] * worig[r][i, q, o]
                # r = 0 has angle rot_alpha[0]*(pi/4)*0 = 0 for ANY input, so
                # M[0] is exactly the identity: just a cast on the idle ACT
                # engine.  Other rotations run as 17 batched broadcast
                # multiply/adds on DVE, explicitly chained in r order so the
                # scheduler cannot interleave chains and delay early
                # rotations.
                wsr = worig[r]
                if r == 0:
                    nc.vector.tensor_copy(
                        rotw[0][:, :, :].rearrange("i q o -> i (q o)"),
                        wsr[:, 0 : 9 * O],
                    )
                    return
                acc = tpool.tile([128, 9, O], F32, name=f"rA{r}", tag="rA")
                tmp = tpool.tile([128, 9, O], F32, name=f"tA{r}", tag="tA")
                first_op = None
                for q in range(9):
                    in0 = wsr[:, None, q * O : (q + 1) * O].broadcast_to([128, 9, O])
                    mcol = wsr[:, 9 * O + q : 9 * O + q + 73 : 9]
                    in1 = mcol[:, :, None].broadcast_to([128, 9, O])
                    if q == 0:
                        op = nc.vector.tensor_tensor(
                            acc[:, :, :], in0, in1, mybir.AluOpType.mult
                        )
                        first_op = op
                    else:
                        nc.vector.tensor_tensor(
                            tmp[:, :, :], in0, in1, mybir.AluOpType.mult
                        )
                        dst = rotw[r] if q == 8 else acc
                        op = nc.vector.tensor_tensor(
                            dst[:, :, :], acc[:, :, :], tmp[:, :, :],
                            mybir.AluOpType.add,
                        )
                if last_rot_op[0] is not None:
                    add_dep_helper(
                        first_op.ins, last_rot_op[0].ins, sync=False,
                        reason="rotations complete in r order",
                    )
                last_rot_op[0] = op

            next_rot = [1]

            # x staging: manual ping-pong between two persistent buffers so
            # the zero padding (columns 0 and W+1, boundary halo rows) is
            # established once instead of re-memset every block.
            nxst = 2 if use_bf16 else 3
            xst2 = [
                xpool.tile([128, RB + 2, W + 2], F32, name=f"xst{i}", tag=f"xst{i}")
                for i in range(nxst)
            ]
            xmm2 = [
                xpool.tile([128, RB + 2, W + 2], BF16, name=f"xmm{i}", tag=f"xmm{i}")
                for i in range(3)
            ] if use_bf16 else xst2
            for i in range(2):
                nc.gpsimd.memset(xst2[i][:, :, :], 0.0)

            def load_x(g, b, blk, chunks=1, cuts=None, first_cast_dve=False):
                # DMA the block's input rows (with halo) into the ping-pong
                # staging buffer and cast to the matmul dtype.  `chunks`
                # splits the load so downstream matmuls can start on the
                # first rows before the whole block has landed.
                h0 = blk * RB
                r0 = max(h0 - 1, 0)
                r1 = min(h0 + RB + 1, H)
                xst = xst2[g % nxst]
                xmm = xmm2[g % 3] if use_bf16 else xst
                if g >= nxst:
                    # restore halo-row zeros clobbered by the previous user
                    # of this buffer (interior blocks write all 34 rows)
                    if blk == 0:
                        nc.gpsimd.memset(xst[:, 0:1, :], 0.0)
                    elif blk == NBLK - 1:
                        nc.gpsimd.memset(xst[:, RB + 1 : RB + 2, :], 0.0)
                d0 = r0 - (h0 - 1)
                nrows = r1 - r0
                if cuts is None:
                    cuts = [nrows * k // chunks for k in range(chunks + 1)]
                for k in range(len(cuts) - 1):
                    a, c = cuts[k], cuts[k + 1]
                    nc.sync.dma_start(
                        out=xst[:, d0 + a : d0 + c, 1 : W + 1],
                        in_=xs[b, :, r0 + a : r0 + c, :],
                    )
                    if use_bf16:
                        # cast range covers the pad rows on the outer chunks
                        ca = d0 + a if k > 0 else 0
                        cc = d0 + c if k < len(cuts) - 2 else RB + 2
                        if k == 0 and first_cast_dve:  # noqa: SIM114
                            # first chunk cast on DVE, ahead of the rotation
                            # chain, so the PE can start within ~12us
                            op = nc.vector.tensor_copy(
                                xmm[:, ca:cc, :], xst[:, ca:cc, :]
                            )
                            last_rot_op[0] = op
                        else:
                            nc.scalar.copy(xmm[:, ca:cc, :], xst[:, ca:cc, :])
                return xmm

            def conv_chunk(xmm, acc, r, store=None, s_groups=1):
                pst = [
                    ppool.tile([128, 4, W], F32, name=f"ps{s}", tag=f"ps{s}")
                    for s in range(NS)
                ]

                def emit_group(ss):
                    for p in range(9):
                        ky, kx = divmod(p, 3)
                        lhsT = rotw[r][:, p, :]
                        if not use_bf16:
                            lhsT = lhsT.bitcast(F32R)
                        for s in ss:
                            rhs = xmm[:, 4 * s + ky : 4 * s + ky + 4, kx : kx + W]
                            if not use_bf16:
                                rhs = rhs.bitcast(F32R)
                            nc.tensor.matmul(
                                pst[s][:, :, :], lhsT, rhs,
                                start=(p == 0), stop=(p == 8),
                            )
                    for s in ss:
                        if r == 0:
                            nc.vector.tensor_copy(
                                acc[:, 4 * s : 4 * s + 4, :], pst[s][:, :, :]
                            )
                        else:
                            nc.vector.tensor_tensor(
                                acc[:, 4 * s : 4 * s + 4, :],
                                acc[:, 4 * s : 4 * s + 4, :],
                                pst[s][:, :, :],
                                mybir.AluOpType.max,
                            )
                        if store is not None:
                            b, h0 = store
                            nc.sync.dma_start(
                                out=y[b, :, h0 + 4 * s : h0 + 4 * s + 4, :],
                                in_=acc[:, 4 * s : 4 * s + 4, :],
                            )

                per = NS // s_groups
                for k in range(s_groups):
                    emit_group(range(k * per, (k + 1) * per))

            # Blocks 0..2 are fused into one r-loop: each rotation r is
            # consumed by three conv chunks (~46us of PE work), giving the
            # DVE rotation pipeline enough slack to stay ahead of the PE.
            # DMA issue order matters (the sync queue issues serially):
            # wsr0 and the first 5 x rows go first so the identity cast and
            # the first matmul tap are unblocked as early as possible.
            nc.sync.dma_start(out=worig[0][:, :], in_=wl[0, :, :])
            emit_rotate(0)
            next_rot[0] = 1
            xmm_first = load_x(0, 0, 0, cuts=[0, 3, 5, 14, 23, 33],
                               first_cast_dve=True)
            nc.sync.dma_start(out=worig[1][:, :], in_=wl[1, :, :])
            xmmF = [xmm_first, load_x(1, 0, 1, chunks=2),
                    load_x(2, 0, 2, chunks=2)]
            for r in range(2, R):
                nc.sync.dma_start(out=worig[r][:, :], in_=wl[r, :, :])
            accF = [apool.tile([128, RB, W], F32, name=f"accF{i}", tag="acc")
                    for i in range(3)]
            for r in range(R):
                for i in range(3):
                    st = (0, i * RB) if r == R - 1 else None
                    conv_chunk(xmmF[i], accF[i], r, store=st)
                    if i == 0 and next_rot[0] < R:
                        emit_rotate(next_rot[0])
                        next_rot[0] += 1

            last_g = BL * NBLK - 1
            for g in range(3, BL * NBLK):
                b, blk = divmod(g, NBLK)
                xmm = load_x(g, b, blk)
                acc = apool.tile([128, RB, W], F32, name="acc", tag="acc")
                for r in range(R):
                    final = r == R - 1
                    conv_chunk(
                        xmm, acc, r,
                        store=(b, blk * RB) if final else None,
                        s_groups=4 if (final and g == last_g) else 1,
                    )
    nc.finalize()
    return nc


def _get_nc():
    if MM_DTYPE not in _NC_CACHE:
        _NC_CACHE[MM_DTYPE] = _build(MM_DTYPE)
    return _NC_CACHE[MM_DTYPE]


def kernel(x, weight, rot_alpha):
    global LAST_RESULTS
    x = np.ascontiguousarray(np.asarray(x, np.float32))
    weight = np.ascontiguousarray(np.asarray(weight, np.float32))
    rot_alpha = np.asarray(rot_alpha, np.float32)

    M = _rot_mats(rot_alpha)
    # wl[r, i, :1152] = weight[o*R + r, i, qy, qx] laid out (q, o);
    # wl[r, i, 1152:] = M[r] flattened (replicated across i).
    wq = weight.reshape(O, R, CIN, 9).transpose(1, 2, 3, 0).reshape(R, CIN, 9 * O)
    mrep = np.broadcast_to(M.reshape(R, 1, 81), (R, CIN, 81))
    wl = np.ascontiguousarray(np.concatenate([wq, mrep], axis=2), dtype=np.float32)

    nc = _get_nc()
    in_maps = [
        {"xs": np.ascontiguousarray(x[c * BL : (c + 1) * BL]), "wl": wl}
        for c in range(NCORES)
    ]
    try:
        res = run_bass_kernel_spmd(nc, in_maps, list(range(NCORES)), trace=_TRACE)
    except Exception:
        # One retry (without tracing): a failed compile or an aborted run can
        # leave a NeuronCore transiently wedged; the next attempt recovers.
        res = run_bass_kernel_spmd(nc, in_maps, list(range(NCORES)), trace=False)
    LAST_RESULTS = res
    return np.concatenate([res.results[c]["y"] for c in range(NCORES)], axis=0)



# revision 3
# speedup vs baseline: 1.3837x; 1.3837x over previous
"""Equivariant rotation conv for Trainium2, 8-core batch-parallel,
vertical-Winograd F(2,3) formulation.

Computes: rotate a (128*8, 128, 3, 3) filter bank by 8 data-dependent angles
(bilinear resampling), run a 3x3 same-padded conv of x (16,128,128,128) with
all 8*128 rotated filters, then max over the 8 rotations -> (16,128,128,128).

Sharding: data-parallel over batch, 2 images per core; the filter bank is
replicated.  The rotation (a 9x9 bilinear mix, a pure function of the 8
rot_alpha scalars) and a vertical Winograd F(2,3) G-transform are folded into
the weights on the host, producing 4 transformed vertical taps x 3 horizontal
taps per rotation in bf16.  On device, per core:
  - x arrives pre-cast to bf16; per 32-row block the DVE builds 4 transformed
    row-planes (t0 = d0-d2, t1 = d1+d2, t2 = d2-d1, t3 = d1-d3 over row pairs)
    with strided-row tensor_tensor ops in the 2x bf16 mode,
  - the conv needs only 12 PE matmuls per 8 output rows (4 m-planes x 3
    horizontal taps, f32 PSUM accumulation) instead of 18 direct ones: the
    two output rows of each pair are recombined as y0 = m0+m1+m2,
    y1 = m1-m2-m3 outside the PE,
  - ACT copies each 4-bank PSUM group to bf16 SBUF in one op; the DVE then
    runs the inverse transform + running rotation max entirely in the 2x
    bf16 mode,
  - ACT expands the bf16 even/odd accumulators to the f32 output staging
    buffer, fused with the per-block output DMA.
"""

import numpy as np
import ml_dtypes


def _install_axon_hooks_shim():
    """Provide antenv.axon_hooks (NTFF profile hook) when the image's antenv
    lacks it, so run_bass_kernel_spmd(trace=True) works instead of crashing
    on import."""
    import contextlib
    import ctypes
    import os
    import sys
    import types

    try:
        import antenv.axon_hooks  # noqa: F401

        return
    except ImportError:
        pass

    state = {"hook": None, "resolved": False}

    def _make_hook():
        so_path = os.environ.get("AXON_PJRT_SO", "/opt/axon/libaxon_pjrt.so")
        if not os.path.exists(so_path):
            return None
        lib = ctypes.CDLL(so_path)
        if not hasattr(lib, "axon_start_nrt_profile"):
            return None
        lib.axon_start_nrt_profile.argtypes = [
            ctypes.POINTER(ctypes.c_int64),
            ctypes.c_size_t,
        ]
        lib.axon_start_nrt_profile.restype = ctypes.c_int64
        lib.axon_stop_nrt_profile.argtypes = [ctypes.c_char_p]
        lib.axon_stop_nrt_profile.restype = ctypes.c_int64

        @contextlib.contextmanager
        def _hook(output_dir, device_ids):
            import jax

            jax.devices()
            if device_ids:
                ids = (ctypes.c_int64 * len(device_ids))(*device_ids)
                rc = lib.axon_start_nrt_profile(ids, len(device_ids))
            else:
                rc = lib.axon_start_nrt_profile(None, 0)
            if rc != 0:
                raise RuntimeError(f"axon_start_nrt_profile rc={rc}")
            try:
                yield
            finally:
                n = lib.axon_stop_nrt_profile(str(output_dir).encode())
                if n < 0:
                    raise RuntimeError(f"axon_stop_nrt_profile rc={n}")
                print(f"profile: {n} file(s) written to {output_dir}")

        return _hook

    mod = types.ModuleType("antenv.axon_hooks")

    def set_axon_ntff_profile_hook(h):
        state["hook"] = h
        state["resolved"] = True

    def get_axon_ntff_profile_hook():
        if not state["resolved"]:
            state["hook"] = _make_hook()
            state["resolved"] = True
        return state["hook"]

    mod.set_axon_ntff_profile_hook = set_axon_ntff_profile_hook
    mod.get_axon_ntff_profile_hook = get_axon_ntff_profile_hook
    sys.modules["antenv.axon_hooks"] = mod


_install_axon_hooks_shim()

import concourse.bass as bass  # noqa: E402,F401
import concourse.mybir as mybir  # noqa: E402
from concourse import bacc  # noqa: E402
from concourse.bass_utils import run_bass_kernel_spmd  # noqa: E402
from concourse.tile import TileContext  # noqa: E402

F32 = mybir.dt.float32
BF16 = mybir.dt.bfloat16
BF16NP = ml_dtypes.bfloat16

B, CIN, H, W = 16, 128, 128, 128
R, O, K = 8, 128, 3
NCORES = 8
BL = B // NCORES   # images per core
RB = 32            # output rows per block
NPAIR = RB // 2    # winograd row pairs per block
NG = NPAIR // 4    # matmul groups (4 pairs = 8 output rows) per block
NBLK = H // RB

ADD = mybir.AluOpType.add
SUB = mybir.AluOpType.subtract
MAX = mybir.AluOpType.max

_TRACE = False
LAST_RESULTS = None
_NC_CACHE = {}


def _rot_mats(rot_alpha):
    """Per-rotation 9x9 bilinear resampling matrices, matching the reference
    F.grid_sample(align_corners=True, zeros) tap logic exactly.

    M[r, p, q]: coefficient of original tap q = (qy*3+qx) in rotated tap
    p = (py*3+px)."""
    M = np.zeros((R, 9, 9), np.float64)
    lin = np.linspace(-1.0, 1.0, K)
    for r in range(R):
        ang = float(rot_alpha[r]) * (np.pi / 4.0) * r
        c, s = np.cos(ang), np.sin(ang)
        for a in range(K):          # output row (gy = lin[a])
            for b in range(K):      # output col (gx = lin[b])
                gx, gy = lin[b], lin[a]
                xs = c * gx - s * gy
                ys = s * gx + c * gy
                ix = (xs + 1.0) * 0.5 * (K - 1)
                iy = (ys + 1.0) * 0.5 * (K - 1)
                x0 = int(np.floor(ix))
                y0 = int(np.floor(iy))
                wx, wy = ix - x0, iy - y0
                p = a * K + b
                for yi, xi, wt in (
                    (y0, x0, (1 - wy) * (1 - wx)),
                    (y0, x0 + 1, (1 - wy) * wx),
                    (y0 + 1, x0, wy * (1 - wx)),
                    (y0 + 1, x0 + 1, wy * wx),
                ):
                    if 0 <= yi < K and 0 <= xi < K:
                        M[r, p, yi * K + xi] += wt
    return M.astype(np.float32)


def _build():
    nc = bacc.Bacc(trn_type="TRN2")
    xs = nc.dram_tensor("xs", [BL, CIN, H, W], BF16, kind="ExternalInput")
    # wt[r, i, (j*3+kx)*O + o]: vertical-Winograd-transformed rotated filters
    wt = nc.dram_tensor("wt", [R, CIN, 12 * O], BF16, kind="ExternalInput")
    y = nc.dram_tensor("y", [BL, O, H, W], F32, kind="ExternalOutput")

    with TileContext(nc) as tc:
        with (
            tc.tile_pool(name="wpool", bufs=1) as wpool,
            tc.tile_pool(name="xpool", bufs=1) as xpool,
            tc.tile_pool(name="cpool", bufs=1) as cpool,
            tc.tile_pool(name="psum", bufs=1, space="PSUM") as ppool,
        ):
            # transformed weights: [cin, r, 12, O], all rotations resident
            wtile = wpool.tile([128, R, 12, O], BF16, name="wtile", tag="wt")

            # PE warm-up: dependency-free matmuls on a scratch tile keep the
            # PE busy from ~0.5us until the first real matmul so the HAM
            # clock gate reaches 8/8 before real work.
            dum_lhs = wpool.tile([128, 128], BF16, name="dum_lhs", tag="dum")
            nc.gpsimd.memset(dum_lhs[:, :], 0.0)
            dum_ps = ppool.tile([128, 128], F32, name="dum_ps", tag="P0")
            for _ in range(125):
                nc.tensor.matmul(
                    dum_ps[:, :], dum_lhs[:, :], dum_lhs[:, :],
                    start=True, stop=True,
                )

            # weight DMA: rotation 0 first so block 0 is unblocked early
            for r in range(R):
                nc.sync.dma_start(out=wtile[:, r, :, :], in_=wt[r, :, :])

            # x staging ping-pong: [34 rows, 130 cols] bf16, halo zeroed once
            xmm2 = [
                xpool.tile([128, RB + 2, W + 2], BF16, name=f"xmm{i}", tag=f"xmm{i}")
                for i in range(2)
            ]
            for i in range(2):
                nc.gpsimd.memset(xmm2[i][:, :, :], 0.0)

            # winograd row planes: [16 pairs, 130] x 4, double buffered
            tst = [
                [
                    xpool.tile([128, NPAIR, W + 2], BF16, name=f"t{p}{j}", tag=f"t{p}{j}")
                    for j in range(4)
                ]
                for p in range(2)
            ]

            def load_x(g, b, blk):
                h0 = blk * RB
                r0 = max(h0 - 1, 0)
                r1 = min(h0 + RB + 1, H)
                xmm = xmm2[g % 2]
                if g >= 2:
                    # restore halo-row zeros clobbered by the previous user
                    if blk == 0:
                        nc.gpsimd.memset(xmm[:, 0:1, :], 0.0)
                    elif blk == NBLK - 1:
                        nc.gpsimd.memset(xmm[:, RB + 1 : RB + 2, :], 0.0)
                d0 = r0 - (h0 - 1)
                nc.sync.dma_start(
                    out=xmm[:, d0 : d0 + (r1 - r0), 1 : W + 1],
                    in_=xs[b, :, r0:r1, :],
                )
                return xmm

            def transform(g, xmm):
                # pair s covers output rows 2s, 2s+1 of the block;
                # d_k = xmm row 2s+k (xmm row i = image row h0-1+i)
                t = tst[g % 2]
                d = [
                    xmm[:, k : min(k + 2 * NPAIR, RB + 2) : 2, :] for k in range(4)
                ]
                nc.vector.tensor_tensor(t[0][:, :, :], d[0], d[2], SUB)
                nc.vector.tensor_tensor(t[1][:, :, :], d[1], d[2], ADD)
                nc.vector.tensor_tensor(t[2][:, :, :], d[2], d[1], SUB)
                nc.vector.tensor_tensor(t[3][:, :, :], d[1], d[3], SUB)

            # psum: 2 phases x [4 m-planes, 4 pairs, W] f32 = 2 x 4 banks
            P = [
                ppool.tile([128, 4, 4, W], F32, name=f"P{p}", tag=f"P{p}")
                for p in range(2)
            ]
            mb = [
                cpool.tile([128, 4, 4, W], BF16, name=f"mb{p}", tag=f"mb{p}")
                for p in range(2)
            ]
            uv = [
                [
                    cpool.tile([128, 4, W], BF16, name=f"uv{p}{i}", tag=f"uv{p}{i}")
                    for i in range(4)
                ]
                for p in range(2)
            ]
            accE = [
                cpool.tile([128, NPAIR, W], BF16, name=f"accE{p}", tag=f"accE{p}")
                for p in range(2)
            ]
            accO = [
                cpool.tile([128, NPAIR, W], BF16, name=f"accO{p}", tag=f"accO{p}")
                for p in range(2)
            ]
            outf = [
                cpool.tile([128, RB, W], F32, name=f"outf{p}", tag=f"outf{p}")
                for p in range(2)
            ]

            gctr = [0]

            def conv_group(g, r, sp):
                ph = gctr[0] % 2
                gctr[0] += 1
                t = tst[g % 2]
                for j in range(4):
                    for kx in range(3):
                        nc.tensor.matmul(
                            P[ph][:, j, :, :],
                            wtile[:, r, j * 3 + kx, :],
                            t[j][:, 4 * sp : 4 * sp + 4, kx : kx + W],
                            start=(kx == 0), stop=(kx == 2),
                        )
                nc.scalar.copy(mb[ph][:, :, :, :], P[ph][:, :, :, :])
                m0, m1 = mb[ph][:, 0], mb[ph][:, 1]
                m2, m3 = mb[ph][:, 2], mb[ph][:, 3]
                u, v, y0, y1 = uv[ph]
                nc.vector.tensor_tensor(u[:, :, :], m0, m1, ADD)
                nc.vector.tensor_tensor(v[:, :, :], m1, m2, SUB)
                aE = accE[g % 2][:, 4 * sp : 4 * sp + 4, :]
                aO = accO[g % 2][:, 4 * sp : 4 * sp + 4, :]
                if r == 0:
                    nc.vector.tensor_tensor(aE, u[:, :, :], m2, ADD)
                    nc.vector.tensor_tensor(aO, v[:, :, :], m3, SUB)
                else:
                    nc.vector.tensor_tensor(y0[:, :, :], u[:, :, :], m2, ADD)
                    nc.vector.tensor_tensor(y1[:, :, :], v[:, :, :], m3, SUB)
                    nc.vector.tensor_tensor(aE, aE, y0[:, :, :], MAX)
                    nc.vector.tensor_tensor(aO, aO, y1[:, :, :], MAX)

            def flush_block(g, b, blk):
                p = g % 2
                h0 = blk * RB
                nc.scalar.copy(outf[p][:, 0:RB:2, :], accE[p][:, :, :])
                nc.scalar.copy(outf[p][:, 1:RB:2, :], accO[p][:, :, :])
                nc.sync.dma_start(
                    out=y[b, :, h0 : h0 + RB, :], in_=outf[p][:, :, :]
                )

            blocks = [(g, divmod(g, NBLK)) for g in range(BL * NBLK)]
            xmm0 = load_x(0, *blocks[0][1])
            transform(0, xmm0)
            for g, (b, blk) in blocks:
                for r in range(R):
                    if r == 1 and g + 1 < len(blocks):
                        nb, nblk = blocks[g + 1][1]
                        transform(g + 1, load_x(g + 1, nb, nblk))
                    for sp in range(NG):
                        conv_group(g, r, sp)
                flush_block(g, b, blk)
    nc.finalize()
    return nc


def _get_nc():
    if "wino" not in _NC_CACHE:
        _NC_CACHE["wino"] = _build()
    return _NC_CACHE["wino"]


def _prep_weights(weight, rot_alpha):
    """Rotate the filter bank by the 8 angles and fold the vertical Winograd
    F(2,3) G-transform in; returns [R, CIN, 12*O] bf16."""
    M = _rot_mats(rot_alpha)
    w_r = (
        weight.reshape(O, R, CIN, 9).transpose(1, 0, 2, 3).astype(np.float64)
    )  # (R, O, I, 9)
    rot = np.einsum("rpq,roiq->roip", M.astype(np.float64), w_r)
    rot = rot.reshape(R, O, CIN, 3, 3)  # (ky, kx)
    G = np.array(
        [[1, 0, 0], [0.5, 0.5, 0.5], [0.5, -0.5, 0.5], [0, 0, 1]], np.float64
    )
    gp = np.einsum("jk,roikx->rijxo", G, rot)  # (R, I, 4, 3, O)
    return np.ascontiguousarray(
        gp.reshape(R, CIN, 12 * O).astype(np.float32).astype(BF16NP)
    )


def kernel(x, weight, rot_alpha):
    global LAST_RESULTS
    x = np.asarray(x, np.float32)
    weight = np.asarray(weight, np.float32)
    rot_alpha = np.asarray(rot_alpha, np.float32)

    wt = _prep_weights(weight, rot_alpha)
    xb = np.ascontiguousarray(x.astype(BF16NP))

    nc = _get_nc()
    in_maps = [
        {"xs": np.ascontiguousarray(xb[c * BL : (c + 1) * BL]), "wt": wt}
        for c in range(NCORES)
    ]
    try:
        res = run_bass_kernel_spmd(nc, in_maps, list(range(NCORES)), trace=_TRACE)
    except Exception:
        # One retry (without tracing): a failed compile or an aborted run can
        # leave a NeuronCore transiently wedged; the next attempt recovers.
        res = run_bass_kernel_spmd(nc, in_maps, list(range(NCORES)), trace=False)
    LAST_RESULTS = res
    return np.concatenate([res.results[c]["y"] for c in range(NCORES)], axis=0)


# revision 6
# speedup vs baseline: 1.4048x; 1.0153x over previous
"""Equivariant rotation conv for Trainium2, 8-core batch-parallel,
vertical-Winograd F(2,3) formulation.

Computes: rotate a (128*8, 128, 3, 3) filter bank by 8 data-dependent angles
(bilinear resampling), run a 3x3 same-padded conv of x (16,128,128,128) with
all 8*128 rotated filters, then max over the 8 rotations -> (16,128,128,128).

Sharding: data-parallel over batch, 2 images per core; the filter bank is
replicated.  The rotation (a 9x9 bilinear mix, a pure function of the 8
rot_alpha scalars) and a vertical Winograd F(2,3) G-transform are folded into
the weights on the host, producing 4 transformed vertical taps x 3 horizontal
taps per rotation in bf16.  On device, per core:
  - x arrives pre-cast to bf16; per 32-row block the DVE builds 4 transformed
    row-planes (t0 = d0-d2, t1 = d1+d2, t2 = d2-d1, t3 = d1-d3 over row pairs)
    with strided-row tensor_tensor ops in the 2x bf16 mode,
  - the conv needs only 12 PE matmuls per 8 output rows (4 m-planes x 3
    horizontal taps, f32 PSUM accumulation) instead of 18 direct ones: the
    two output rows of each pair are recombined as y0 = m0+m1+m2,
    y1 = m1-m2-m3 outside the PE,
  - ACT copies each 4-bank PSUM group to bf16 SBUF in one op; the DVE then
    runs the inverse transform + running rotation max entirely in the 2x
    bf16 mode,
  - ACT expands the bf16 even/odd accumulators to the f32 output staging
    buffer, fused with the per-block output DMA.
"""

import numpy as np
import ml_dtypes


def _install_axon_hooks_shim():
    """Provide antenv.axon_hooks (NTFF profile hook) when the image's antenv
    lacks it, so run_bass_kernel_spmd(trace=True) works instead of crashing
    on import."""
    import contextlib
    import ctypes
    import os
    import sys
    import types

    try:
        import antenv.axon_hooks  # noqa: F401

        return
    except ImportError:
        pass

    state = {"hook": None, "resolved": False}

    def _make_hook():
        so_path = os.environ.get("AXON_PJRT_SO", "/opt/axon/libaxon_pjrt.so")
        if not os.path.exists(so_path):
            return None
        lib = ctypes.CDLL(so_path)
        if not hasattr(lib, "axon_start_nrt_profile"):
            return None
        lib.axon_start_nrt_profile.argtypes = [
            ctypes.POINTER(ctypes.c_int64),
            ctypes.c_size_t,
        ]
        lib.axon_start_nrt_profile.restype = ctypes.c_int64
        lib.axon_stop_nrt_profile.argtypes = [ctypes.c_char_p]
        lib.axon_stop_nrt_profile.restype = ctypes.c_int64

        @contextlib.contextmanager
        def _hook(output_dir, device_ids):
            import jax

            jax.devices()
            if device_ids:
                ids = (ctypes.c_int64 * len(device_ids))(*device_ids)
                rc = lib.axon_start_nrt_profile(ids, len(device_ids))
            else:
                rc = lib.axon_start_nrt_profile(None, 0)
            if rc != 0:
                raise RuntimeError(f"axon_start_nrt_profile rc={rc}")
            try:
                yield
            finally:
                n = lib.axon_stop_nrt_profile(str(output_dir).encode())
                if n < 0:
                    raise RuntimeError(f"axon_stop_nrt_profile rc={n}")
                print(f"profile: {n} file(s) written to {output_dir}")

        return _hook

    mod = types.ModuleType("antenv.axon_hooks")

    def set_axon_ntff_profile_hook(h):
        state["hook"] = h
        state["resolved"] = True

    def get_axon_ntff_profile_hook():
        if not state["resolved"]:
            state["hook"] = _make_hook()
            state["resolved"] = True
        return state["hook"]

    mod.set_axon_ntff_profile_hook = set_axon_ntff_profile_hook
    mod.get_axon_ntff_profile_hook = get_axon_ntff_profile_hook
    sys.modules["antenv.axon_hooks"] = mod


_install_axon_hooks_shim()

import concourse.bass as bass  # noqa: E402,F401
import concourse.mybir as mybir  # noqa: E402
from concourse import bacc  # noqa: E402
from concourse.bass_utils import run_bass_kernel_spmd  # noqa: E402
from concourse.tile import TileContext  # noqa: E402

F32 = mybir.dt.float32
BF16 = mybir.dt.bfloat16
BF16NP = ml_dtypes.bfloat16

B, CIN, H, W = 16, 128, 128, 128
R, O, K = 8, 128, 3
NCORES = 8
BL = B // NCORES   # images per core
RB = 32            # output rows per block
NPAIR = RB // 2    # winograd row pairs per block
NG = NPAIR // 4    # matmul groups (4 pairs = 8 output rows) per block
NBLK = H // RB

ADD = mybir.AluOpType.add
SUB = mybir.AluOpType.subtract
MAX = mybir.AluOpType.max

_TRACE = False
LAST_RESULTS = None
_NC_CACHE = {}


def _rot_mats(rot_alpha):
    """Per-rotation 9x9 bilinear resampling matrices, matching the reference
    F.grid_sample(align_corners=True, zeros) tap logic exactly.

    M[r, p, q]: coefficient of original tap q = (qy*3+qx) in rotated tap
    p = (py*3+px)."""
    M = np.zeros((R, 9, 9), np.float64)
    lin = np.linspace(-1.0, 1.0, K)
    for r in range(R):
        ang = float(rot_alpha[r]) * (np.pi / 4.0) * r
        c, s = np.cos(ang), np.sin(ang)
        for a in range(K):          # output row (gy = lin[a])
            for b in range(K):      # output col (gx = lin[b])
                gx, gy = lin[b], lin[a]
                xs = c * gx - s * gy
                ys = s * gx + c * gy
                ix = (xs + 1.0) * 0.5 * (K - 1)
                iy = (ys + 1.0) * 0.5 * (K - 1)
                x0 = int(np.floor(ix))
                y0 = int(np.floor(iy))
                wx, wy = ix - x0, iy - y0
                p = a * K + b
                for yi, xi, wt in (
                    (y0, x0, (1 - wy) * (1 - wx)),
                    (y0, x0 + 1, (1 - wy) * wx),
                    (y0 + 1, x0, wy * (1 - wx)),
                    (y0 + 1, x0 + 1, wy * wx),
                ):
                    if 0 <= yi < K and 0 <= xi < K:
                        M[r, p, yi * K + xi] += wt
    return M.astype(np.float32)


def _build():
    nc = bacc.Bacc(trn_type="TRN2")
    xs = nc.dram_tensor("xs", [BL, CIN, H, W], BF16, kind="ExternalInput")
    # wt[r, i, (j*3+kx)*O + o]: vertical-Winograd-transformed rotated filters
    wt = nc.dram_tensor("wt", [R, CIN, 12 * O], BF16, kind="ExternalInput")
    y = nc.dram_tensor("y", [BL, O, H, W], F32, kind="ExternalOutput")

    with TileContext(nc) as tc:
        with (
            tc.tile_pool(name="wpool", bufs=1) as wpool,
            tc.tile_pool(name="xpool", bufs=1) as xpool,
            tc.tile_pool(name="cpool", bufs=1) as cpool,
            tc.tile_pool(name="psum", bufs=1, space="PSUM") as ppool,
        ):
            # transformed weights: [cin, r, 12, O], all rotations resident
            wtile = wpool.tile([128, R, 12, O], BF16, name="wtile", tag="wt")

            # PE warm-up: dependency-free matmuls on a scratch tile keep the
            # PE busy from ~0.5us until the first real matmul so the HAM
            # clock gate reaches 8/8 before real work.
            dum_lhs = wpool.tile([128, 128], BF16, name="dum_lhs", tag="dum")
            nc.gpsimd.memset(dum_lhs[:, :], 0.0)
            dum_ps = ppool.tile([128, 128], F32, name="dum_ps", tag="P0")
            for _ in range(125):
                nc.tensor.matmul(
                    dum_ps[:, :], dum_lhs[:, :], dum_lhs[:, :],
                    start=True, stop=True,
                )

            # weight DMA: rotation 0 first so block 0 is unblocked early
            for r in range(R):
                nc.sync.dma_start(out=wtile[:, r, :, :], in_=wt[r, :, :])

            # x staging ping-pong: [34 rows, 130 cols] bf16, halo zeroed once
            xmm2 = [
                xpool.tile([128, RB + 2, W + 2], BF16, name=f"xmm{i}", tag=f"xmm{i}")
                for i in range(2)
            ]
            for i in range(2):
                nc.gpsimd.memset(xmm2[i][:, :, :], 0.0)

            # winograd row planes: [16 pairs, 130] x 4, double buffered
            tst = [
                [
                    xpool.tile([128, NPAIR, W + 2], BF16, name=f"t{p}{j}", tag=f"t{p}{j}")
                    for j in range(4)
                ]
                for p in range(2)
            ]

            def load_x(g, b, blk, chunks=None):
                h0 = blk * RB
                r0 = max(h0 - 1, 0)
                r1 = min(h0 + RB + 1, H)
                xmm = xmm2[g % 2]
                if g >= 2:
                    # restore halo-row zeros clobbered by the previous user
                    if blk == 0:
                        nc.gpsimd.memset(xmm[:, 0:1, :], 0.0)
                    elif blk == NBLK - 1:
                        nc.gpsimd.memset(xmm[:, RB + 1 : RB + 2, :], 0.0)
                d0 = r0 - (h0 - 1)
                cuts = [0, r1 - r0] if chunks is None else chunks
                for k in range(len(cuts) - 1):
                    a, c = cuts[k], cuts[k + 1]
                    nc.sync.dma_start(
                        out=xmm[:, d0 + a : d0 + c, 1 : W + 1],
                        in_=xs[b, :, r0 + a : r0 + c, :],
                    )
                return xmm

            def transform(g, xmm, pair0=0, pair1=NPAIR):
                # pair s covers output rows 2s, 2s+1 of the block;
                # d_k = xmm row 2s+k (xmm row i = image row h0-1+i)
                t = tst[g % 2]
                d = [
                    xmm[:, 2 * pair0 + k : min(2 * pair1 + k, RB + 2) : 2, :]
                    for k in range(4)
                ]
                sl = slice(pair0, pair1)
                nc.vector.tensor_tensor(t[0][:, sl, :], d[0], d[2], SUB)
                nc.vector.tensor_tensor(t[1][:, sl, :], d[1], d[2], ADD)
                nc.vector.tensor_tensor(t[2][:, sl, :], d[2], d[1], SUB)
                nc.vector.tensor_tensor(t[3][:, sl, :], d[1], d[3], SUB)

            # psum: 2 phases x [4 m-planes, 4 pairs, W] f32 = 2 x 4 banks
            P = [
                ppool.tile([128, 4, 4, W], F32, name=f"P{p}", tag=f"P{p}")
                for p in range(2)
            ]
            mb = [
                cpool.tile([128, 4, 4, W], BF16, name=f"mb{p}", tag=f"mb{p}")
                for p in range(2)
            ]
            uv = [
                [
                    cpool.tile([128, 4, W], BF16, name=f"uv{p}{i}", tag=f"uv{p}{i}")
                    for i in range(2)
                ]
                for p in range(2)
            ]
            # y-pair staging interleaved even/odd, so one fused max per group
            yI = [
                cpool.tile([128, 4, 2, W], BF16, name=f"yI{p}", tag=f"yI{p}")
                for p in range(2)
            ]
            # block accumulator, rows already in output order
            accI = [
                cpool.tile([128, NPAIR, 2, W], BF16, name=f"accI{p}", tag=f"accI{p}")
                for p in range(2)
            ]
            outf = [
                cpool.tile([128, RB, W], F32, name=f"outf{p}", tag=f"outf{p}")
                for p in range(2)
            ]

            gctr = [0]

            def conv_group(g, r, sp):
                ph = gctr[0] % 2
                gctr[0] += 1
                t = tst[g % 2]
                for j in range(4):
                    for kx in range(3):
                        nc.tensor.matmul(
                            P[ph][:, j, :, :],
                            wtile[:, r, j * 3 + kx, :],
                            t[j][:, 4 * sp : 4 * sp + 4, kx : kx + W],
                            start=(kx == 0), stop=(kx == 2),
                        )
                nc.scalar.copy(mb[ph][:, :, :, :], P[ph][:, :, :, :])
                m0, m1 = mb[ph][:, 0], mb[ph][:, 1]
                m2, m3 = mb[ph][:, 2], mb[ph][:, 3]
                u, v = uv[ph]
                acc = accI[g % 2][:, 4 * sp : 4 * sp + 4, :, :]
                yt = acc if r == 0 else yI[ph]
                nc.vector.tensor_tensor(u[:, :, :], m0, m1, ADD)
                nc.vector.tensor_tensor(v[:, :, :], m1, m2, SUB)
                nc.vector.tensor_tensor(yt[:, :, 0, :], u[:, :, :], m2, ADD)
                nc.vector.tensor_tensor(yt[:, :, 1, :], v[:, :, :], m3, SUB)
                if r > 0:
                    nc.vector.tensor_tensor(acc, acc, yt[:, :, :, :], MAX)

            def flush_block(g, b, blk):
                p = g % 2
                h0 = blk * RB
                nc.scalar.copy(
                    outf[p][:, :, :],
                    accI[p][:, :, :, :].rearrange("i s e w -> i (s e) w"),
                )
                nc.sync.dma_start(
                    out=y[b, :, h0 : h0 + RB, :], in_=outf[p][:, :, :]
                )

            blocks = [(g, divmod(g, NBLK)) for g in range(BL * NBLK)]
            # first block: land the first 11 rows early so transform+matmuls
            # for the leading pairs start before the whole block arrives
            xmm0 = load_x(0, *blocks[0][1], chunks=[0, 10, 33])
            transform(0, xmm0, 0, 4)
            transform(0, xmm0, 4, NPAIR)
            for g, (b, blk) in blocks:
                for r in range(R):
                    if r == 1 and g + 1 < len(blocks):
                        nb, nblk = blocks[g + 1][1]
                        transform(g + 1, load_x(g + 1, nb, nblk))
                    if r == 2 and g > 0:
                        flush_block(g - 1, *blocks[g - 1][1])
                    for sp in range(NG):
                        conv_group(g, r, sp)
            flush_block(blocks[-1][0], *blocks[-1][1])
    nc.finalize()
    return nc


def _get_nc():
    if "wino" not in _NC_CACHE:
        _NC_CACHE["wino"] = _build()
    return _NC_CACHE["wino"]


def _prep_weights(weight, rot_alpha):
    """Rotate the filter bank by the 8 angles and fold the vertical Winograd
    F(2,3) G-transform in; returns [R, CIN, 12*O] bf16."""
    M = _rot_mats(rot_alpha)
    w_r = (
        weight.reshape(O, R, CIN, 9).transpose(1, 0, 2, 3).astype(np.float64)
    )  # (R, O, I, 9)
    rot = np.einsum("rpq,roiq->roip", M.astype(np.float64), w_r)
    rot = rot.reshape(R, O, CIN, 3, 3)  # (ky, kx)
    G = np.array(
        [[1, 0, 0], [0.5, 0.5, 0.5], [0.5, -0.5, 0.5], [0, 0, 1]], np.float64
    )
    gp = np.einsum("jk,roikx->rijxo", G, rot)  # (R, I, 4, 3, O)
    return np.ascontiguousarray(
        gp.reshape(R, CIN, 12 * O).astype(np.float32).astype(BF16NP)
    )


def kernel(x, weight, rot_alpha):
    global LAST_RESULTS
    x = np.asarray(x, np.float32)
    weight = np.asarray(weight, np.float32)
    rot_alpha = np.asarray(rot_alpha, np.float32)

    wt = _prep_weights(weight, rot_alpha)
    xb = np.ascontiguousarray(x.astype(BF16NP))

    nc = _get_nc()
    in_maps = [
        {"xs": np.ascontiguousarray(xb[c * BL : (c + 1) * BL]), "wt": wt}
        for c in range(NCORES)
    ]
    try:
        res = run_bass_kernel_spmd(nc, in_maps, list(range(NCORES)), trace=_TRACE)
    except Exception:
        # One retry (without tracing): a failed compile or an aborted run can
        # leave a NeuronCore transiently wedged; the next attempt recovers.
        res = run_bass_kernel_spmd(nc, in_maps, list(range(NCORES)), trace=False)
    LAST_RESULTS = res
    return np.concatenate([res.results[c]["y"] for c in range(NCORES)], axis=0)
